# revision 38
# baseline (speedup 1.0000x reference)
"""DAWNBlock Trainium2 kernel (8 NeuronCores, SPMD, single NEFF launch).

Sharding: tokens split over cores as (batch b = c//2, seq-half hf = c%2),
512 tokens per core. Attention is sharded by (batch, head-group): after a
pair AllGather of Q^T/K^T/V each core runs causal attention for 8 heads over
the full 1024-token sequence of its batch; a second pair AllGather exchanges
attn^T so each core projects (W_O) only its own 512 tokens. The knowledge
stage is token-parallel: knowledge_K^T is streamed through SBUF in bf16,
scores are computed in 32 chunks of 1024, and top-8 selection uses the
hardware max8 instruction over packed floats (bf16 score in the high 16
bits, global index in the low 15 mantissa bits), followed by per-partition
indirect-DMA row gathers from knowledge_V.

Host->device traffic is minimized by shipping every replicated weight as a
1/8 shard and reassembling on-device with DRAM AllGathers: a small-weights
AllGather (compress neurons, W_O, W_QKV, routers) issued at kernel start,
and knowledge_K^T / knowledge_V AllGathers issued after the attention
collectives so the attention pair-AllGathers don't queue behind the 64MB
knowledge-table gather on the in-order collective queue.

Core-parity-dependent data movement (which half of the AllGather output
belongs to this core) is handled with register-backed dynamic DMA slices
(bass.ds) driven by a tiny per-core offsets input, so all 8 cores share one
instruction stream.
"""
import functools
import numpy as np
import ml_dtypes

import concourse.bass as bass
import concourse.bacc as bacc
import concourse.mybir as mybir
import concourse.tile as tile
from concourse.bass_utils import run_bass_kernel_spmd

F32 = mybir.dt.float32
BF16 = mybir.dt.bfloat16
F8 = mybir.dt.float8e4
U32 = mybir.dt.uint32
U16 = mybir.dt.uint16
KV_SCALE = 64.0        # knowledge_V shipped as fp8e4 * KV_SCALE
KK_SCALE = 64.0        # knowledge_K shipped as fp8e4 * KK_SCALE
AF = mybir.ActivationFunctionType
OP = mybir.AluOpType
AX = mybir.AxisListType

N_CORES = 8
P = 128
D = 1024
R = 128
NCMP = 16
NK = 32768
KK = 8
S = 1024
B = 4
TOK = 512
NT = TOK // P          # 4 token tiles per core
EPS = 1e-5
NEG = -1.0e30
KC = 1024              # knowledge-score chunk width
NKC = NK // KC         # 32 chunks
NKSH = NK // N_CORES   # 4096 knowledge rows per core shard
SCALE_R = float(1.0 / np.sqrt(R))

# small-weights AllGather segment layout (element offsets, bf16).
# Split into three collectives so S1 can start after A (routers + first
# half of the neuron banks), B lands just in time for the second compress
# half, and W_O (C) gathers during attention.
RQ_SZ = P * 48                  # 6144
RM_SZ = P * NCMP                # 2048
WX_SZ = 16 * D                  # 16384 (W_Q/W_K/W_V row shard)
NEURH_SZ = P * 1024             # 131072 (half of the neuron banks)
WO_SZ = P * D                   # 131072
RQ0 = 0
RM0 = RQ0 + RQ_SZ
WQ0 = RM0 + RM_SZ
WK0 = WQ0 + WX_SZ
WV0 = WK0 + WX_SZ
NA0 = WV0 + WX_SZ
AGAN = NA0 + NEURH_SZ           # 188416 elems
AGBN = NEURH_SZ                 # 131072
AGCN = WO_SZ                    # 131072


def _ln(nc, sb, x_ap, out_ap, eps_tile):
    """LayerNorm (gamma=1, beta=0): x_ap [128, D] f32 -> out_ap (bf16)."""
    stats = sb.tile([P, 2, 6], F32, tag="ln_stats")
    for g in range(2):
        nc.vector.bn_stats(out=stats[:, g, :], in_=x_ap[:, g * 512:(g + 1) * 512])
    mv = sb.tile([P, 2], F32, tag="ln_mv")
    nc.vector.bn_aggr(out=mv[:], in_=stats[:])
    rstd = sb.tile([P, 1], F32, tag="ln_rstd")
    nc.scalar.activation(out=rstd[:], in_=mv[:, 1:2], func=AF.Sqrt,
                         bias=eps_tile[:], scale=1.0)
    nc.vector.reciprocal(out=rstd[:], in_=rstd[:])
    nc.vector.tensor_scalar(out=out_ap, in0=x_ap, scalar1=mv[:, 0:1],
                            scalar2=rstd[:], op0=OP.subtract, op1=OP.mult)


def _softmax16(nc, sb, logits_ap, w_ap):
    """softmax over 16 router logits (PSUM f32 in) -> w_ap [128,16] f32."""
    mx = sb.tile([P, 1], F32, tag="rs_mx")
    nc.vector.tensor_reduce(out=mx[:], in_=logits_ap, axis=AX.X, op=OP.max)
    nmx = sb.tile([P, 1], F32, tag="rs_nmx")
    nc.vector.tensor_scalar_mul(out=nmx[:], in0=mx[:], scalar1=-1.0)
    ssum = sb.tile([P, 1], F32, tag="rs_sum")
    nc.scalar.activation(out=w_ap, in_=logits_ap, func=AF.Exp,
                         bias=nmx[:], scale=1.0, accum_out=ssum[:])
    nc.vector.reciprocal(out=ssum[:], in_=ssum[:])
    nc.vector.tensor_scalar_mul(out=w_ap, in0=w_ap, scalar1=ssum[:])


def _combine(nc, sb, p1_halves, w_ap, out_ap):
    """out[t,:] = sum_n w[t,n] * P1[t, n*128:(n+1)*128] (P1 in 2 PSUM halves)."""
    acc = sb.tile([P, R], F32, tag="cmb_acc")
    for n in range(NCMP):
        src = p1_halves[n // 8][:, (n % 8) * R:(n % 8 + 1) * R]
        if n == 0:
            nc.vector.tensor_scalar(out=acc[:], in0=src, scalar1=w_ap[:, 0:1],
                                    scalar2=None, op0=OP.mult)
        else:
            nc.vector.scalar_tensor_tensor(out=acc[:], in0=src,
                                           scalar=w_ap[:, n:n + 1], in1=acc[:],
                                           op0=OP.mult, op1=OP.add)
    nc.vector.tensor_copy(out=out_ap, in_=acc[:])


def build_program():
    nc = bacc.Bacc(None, num_devices=N_CORES)

    x_in = nc.dram_tensor("x_shard", [TOK, D], BF16, kind="ExternalInput")
    tri_in = nc.dram_tensor("tri", [P, P], F32, kind="ExternalInput")
    id_in = nc.dram_tensor("ident", [P, P], BF16, kind="ExternalInput")
    offs_in = nc.dram_tensor("offs", [1, 1], U32, kind="ExternalInput")
    aga_in = nc.dram_tensor("agA", [1, AGAN], BF16, kind="ExternalInput")
    agb_in = nc.dram_tensor("agB", [1, AGBN], BF16, kind="ExternalInput")
    agc_in = nc.dram_tensor("agC", [1, AGCN], BF16, kind="ExternalInput")
    kkt_in = nc.dram_tensor("kKTsh", [P, NKSH], F8, kind="ExternalInput")
    kv_in = nc.dram_tensor("kVsh", [NKSH, D], F8, kind="ExternalInput")
    out_t = nc.dram_tensor("out_shard", [TOK, D], BF16, kind="ExternalOutput")

    G8 = [list(range(N_CORES))]

    with tile.TileContext(nc) as tc:
        with (
            tc.tile_pool(name="persist", bufs=1) as pp,
            tc.tile_pool(name="weights", bufs=1) as wp,
            tc.tile_pool(name="work", bufs=2) as sb,
            tc.tile_pool(name="gath", bufs=3) as gp,
            tc.tile_pool(name="ps_big", bufs=2, space="PSUM") as psb,
            tc.tile_pool(name="ps_tp", bufs=2, space="PSUM") as pst,
            tc.tile_pool(name="ps_sm", bufs=2, space="PSUM") as psa,
            tc.tile_pool(name="dram", bufs=1, space="DRAM") as dram,
        ):
            # ---- parity offset -> gpsimd register for dynamic DMA slices ----
            r512 = nc.gpsimd.alloc_register("off512")
            nc.gpsimd.reg_load(r512, offs_in[0:1, 0:1])
            off512 = nc.gpsimd.snap(r512, donate=True, min_val=0, max_val=512)

            # ---- collective staging: bounce shards into DRAM tiles ----
            agab = dram.tile([1, AGAN], BF16)
            agao = dram.tile([N_CORES, AGAN], BF16, addr_space="Shared")
            agbb = dram.tile([1, AGBN], BF16)
            agbo = dram.tile([N_CORES, AGBN], BF16, addr_space="Shared")
            agcb = dram.tile([1, AGCN], BF16)
            agco = dram.tile([N_CORES, AGCN], BF16, addr_space="Shared")
            kktb = dram.tile([P, NKSH], F8)
            kkto = dram.tile([N_CORES * P, NKSH], F8, addr_space="Shared")
            kvb = dram.tile([NKSH, D], F8)
            kvo = dram.tile([NK, D], F8, addr_space="Shared")
            nc.sync.dma_start(out=agab[:], in_=aga_in[:])
            nc.sync.dma_start(out=agbb[:], in_=agb_in[:])
            nc.sync.dma_start(out=agcb[:], in_=agc_in[:])
            # weights gathered in three pieces: S1 needs A (then B); W_O (C)
            # arrives during attention
            nc.gpsimd.collective_compute("AllGather", OP.bypass,
                                         replica_groups=G8,
                                         ins=[agab.opt()], outs=[agao.opt()])
            nc.gpsimd.collective_compute("AllGather", OP.bypass,
                                         replica_groups=G8,
                                         ins=[agbb.opt()], outs=[agbo.opt()])
            nc.gpsimd.collective_compute("AllGather", OP.bypass,
                                         replica_groups=G8,
                                         ins=[agcb.opt()], outs=[agco.opt()])

            # ---- early loads that depend on nothing: masks, x tiles ----
            # (created before the collective-gated weight loads so the
            # in-order SP DMA queue doesn't stall them behind AGb/AGc)
            ident = wp.tile([P, P], BF16)
            nc.sync.dma_start(out=ident[:], in_=id_in[:])
            tri = wp.tile([P, P], F32)
            nc.sync.dma_start(out=tri[:], in_=tri_in[:])
            eps_t = wp.tile([P, 1], F32)
            nc.vector.memset(eps_t[:], EPS)
            iota_t = wp.tile([P, KC], U16)
            nc.gpsimd.iota(out=iota_t[:], pattern=[[64, KC]], base=0,
                           channel_multiplier=0)
            x_all = pp.tile([P, NT, D], F32)
            for t in range(NT):
                xbf = sb.tile([P, D], BF16, tag="xbf")
                nc.sync.dma_start(out=xbf[:], in_=x_in[t * P:(t + 1) * P, :])
                nc.scalar.activation(out=x_all[:, t, :], in_=xbf[:], func=AF.Copy)
            # knowledge-table shard bounces (consumed much later)
            nc.sync.dma_start(out=kktb[:], in_=kkt_in[:])
            nc.sync.dma_start(out=kvb[:], in_=kv_in[:])

            # ---- resident weights (from the gathered small-weights buffers) ----
            neur = wp.tile([P, 8, NCMP * R], BF16)
            nc.sync.dma_start(
                out=neur[:, :, 0:1024],
                in_=agao[:, NA0:NA0 + NEURH_SZ].rearrange(
                    "c (p n) -> p c n", p=P))
            nc.sync.dma_start(
                out=neur[:, :, 1024:2048],
                in_=agbo[:, :].rearrange("c (p n) -> p c n", p=P))
            rQKV = wp.tile([P, 8, 48], BF16)
            nc.sync.dma_start(
                out=rQKV[:],
                in_=agao[:, RQ0:RQ0 + RQ_SZ].rearrange("c (p n) -> p c n", p=P))
            rM = wp.tile([P, 8, NCMP], BF16)
            nc.sync.dma_start(
                out=rM[:],
                in_=agao[:, RM0:RM0 + RM_SZ].rearrange("c (p n) -> p c n", p=P))
            wq = wp.tile([P, D], BF16)
            wk = wp.tile([P, D], BF16)
            wv = wp.tile([P, D], BF16)
            for w_, base in ((wq, WQ0), (wk, WK0), (wv, WV0)):
                for c in range(N_CORES):
                    nc.sync.dma_start(
                        out=w_[c * 16:(c + 1) * 16, :],
                        in_=agao[c:c + 1, base:base + WX_SZ].rearrange(
                            "o (p n) -> (o p) n", p=16))
            wo = wp.tile([P, 8, D], BF16)
            nc.sync.dma_start(
                out=wo[:],
                in_=agco[:, :].rearrange("c (p n) -> p c n", p=P))

            # ---- persistent activations ----
            hT = pp.tile([P, 8, TOK], BF16, tag="hT")
            hQT = pp.tile([P, TOK], BF16, tag="hQT")
            hKT = pp.tile([P, TOK], BF16, tag="hKT")
            hVT = pp.tile([P, TOK], BF16, tag="hVT")

            # packed score buffers (iota pre-written into the low u16 lanes)
            packed = [pp.tile([P, KC], U32, tag=f"pk{i}", name=f"pk{i}") for i in range(3)]
            for pk in packed:
                nc.vector.tensor_copy(out=pk.bitcast(U16)[:, 0::2], in_=iota_t[:])

            # =========== S1: LN1, shared projection, routed compress ===========
            for t in range(NT):
                ts = slice(t * P, (t + 1) * P)
                h = sb.tile([P, D], BF16, tag="h")
                _ln(nc, sb, x_all[:, t, :], h[:], eps_t)
                for ch in range(8):
                    tp = pst.tile([P, P], BF16, tag="tp")
                    nc.tensor.transpose(out=tp[:], in_=h[:, ch * P:(ch + 1) * P],
                                        identity=ident[:])
                    nc.scalar.activation(out=hT[:, ch, ts], in_=tp[:], func=AF.Copy)
                lg = psa.tile([P, 48], F32, tag="sm")
                for ch in range(8):
                    nc.tensor.matmul(out=lg[:], lhsT=hT[:, ch, ts], rhs=rQKV[:, ch, :],
                                     start=(ch == 0), stop=(ch == 7))
                wQKV = sb.tile([P, 48], F32, tag="wQKV")
                for rr in range(3):
                    _softmax16(nc, sb, lg[:, rr * 16:(rr + 1) * 16],
                               wQKV[:, rr * 16:(rr + 1) * 16])
                p1a = psb.tile([P, KC], F32, tag="big")
                p1b = psb.tile([P, KC], F32, tag="big")
                for half, pt in ((0, p1a), (1, p1b)):
                    for col in range(2):
                        c0 = half * KC + col * 512
                        for ch in range(8):
                            nc.tensor.matmul(out=pt[:, col * 512:(col + 1) * 512],
                                             lhsT=hT[:, ch, ts],
                                             rhs=neur[:, ch, c0:c0 + 512],
                                             start=(ch == 0), stop=(ch == 7))
                for rr, dst in ((0, hQT), (1, hKT), (2, hVT)):
                    hc = sb.tile([P, R], BF16, tag="hc")
                    _combine(nc, sb, (p1a, p1b), wQKV[:, rr * 16:(rr + 1) * 16], hc[:])
                    tp = pst.tile([P, P], BF16, tag="tp")
                    nc.tensor.transpose(out=tp[:], in_=hc[:], identity=ident[:])
                    nc.scalar.activation(out=dst[:, ts], in_=tp[:], func=AF.Copy)

            # =========== S3: pair AllGather of h-compressions hQ/hK/hV^T ======
            # Exchange the rank-128 compressed activations (0.38 MB) instead
            # of the projected Q^T/K^T/V (3 MB); both halves' projections are
            # recomputed locally (a few matmuls). The head-half selection is
            # done once by dynamically slicing W_Q/W_K/W_V columns with the
            # parity register, keeping one SPMD instruction stream.
            groups = [[0, 1], [2, 3], [4, 5], [6, 7]]
            xin = dram.tile([P, 1536], BF16)
            xout = dram.tile([2 * P, 1536], BF16)
            nc.sync.dma_start(out=xin[:, 0:512], in_=hQT[:])
            nc.sync.dma_start(out=xin[:, 512:1024], in_=hKT[:])
            nc.sync.dma_start(out=xin[:, 1024:1536], in_=hVT[:])
            nc.gpsimd.collective_compute("AllGather", OP.bypass,
                                         replica_groups=groups,
                                         ins=[xin.opt()], outs=[xout.opt()])
            # parity-sliced projection weights (my 4 head-chunks = 512 cols)
            wq_my = wp.tile([P, TOK], BF16)
            wk_my = wp.tile([P, TOK], BF16)
            wv_my = wp.tile([P, TOK], BF16)
            for w_, wm in ((wq, wq_my), (wk, wk_my), (wv, wv_my)):
                nc.gpsimd.dma_start(out=wm[:],
                                    in_=w_[:][:, bass.ds(off512, TOK)])
            # full-sequence h-compressions (row block s = seq half s)
            hQf = pp.tile([P, S], BF16, tag="hQf")
            hKf = pp.tile([P, S], BF16, tag="hKf")
            hVf = pp.tile([P, S], BF16, tag="hVf")
            for s in range(2):
                rs = slice(s * P, (s + 1) * P)
                sl = slice(s * TOK, (s + 1) * TOK)
                nc.sync.dma_start(out=hQf[:, sl], in_=xout[rs, 0:512])
                nc.sync.dma_start(out=hKf[:, sl], in_=xout[rs, 512:1024])
                nc.sync.dma_start(out=hVf[:, sl], in_=xout[rs, 1024:1536])
            # project both halves for my heads only
            QT_f = pp.tile([P, 4, S], BF16, tag="qt")
            KT_f = pp.tile([P, 4, S], BF16, tag="kt")
            V_f = pp.tile([P, 8, 512], BF16, tag="vv")
            for wm, hsrc, dst in ((wq_my, hQf, QT_f), (wk_my, hKf, KT_f)):
                for i in range(4):
                    pr = psb.tile([P, S], F32, tag="big")
                    for col in range(2):
                        nc.tensor.matmul(out=pr[:, col * 512:(col + 1) * 512],
                                         lhsT=wm[:, i * P:(i + 1) * P],
                                         rhs=hsrc[:, col * 512:(col + 1) * 512],
                                         start=True, stop=True)
                    nc.scalar.activation(out=dst[:, i, :], in_=pr[:], func=AF.Copy)
            for blk in range(8):
                pv = pst.tile([P, TOK], F32, tag="tp")
                nc.tensor.matmul(out=pv[:], lhsT=hVf[:, blk * P:(blk + 1) * P],
                                 rhs=wv_my[:], start=True, stop=True)
                nc.scalar.activation(out=V_f[:, blk, :], in_=pv[:], func=AF.Copy)

            # scheduler fence: the kKT AllGather may not be hoisted before the
            # S3 collective above; it then fills the collective-engine gap
            # while attention computes.
            tc.no_sync_barrier()
            nc.gpsimd.collective_compute("AllGather", OP.bypass,
                                         replica_groups=G8,
                                         ins=[kktb.opt()], outs=[kkto.opt()])

            # =========== S4: causal attention, 8 heads, full sequence ===========
            attnT = pp.tile([P, 4, S], BF16, tag="at")
            for hh in range(8):
                ch, poff = hh // 2, (hh % 2) * 64
                prow = slice(poff, poff + 64)
                for qg in range(8):
                    kr = (qg + 1) * P
                    sc = psb.tile([P, S], F32, tag="big")
                    for part in range((kr + 511) // 512):
                        k0, k1 = part * 512, min(kr, (part + 1) * 512)
                        nc.tensor.matmul(out=sc[:, k0:k1],
                                         lhsT=QT_f[prow, ch, qg * P:(qg + 1) * P],
                                         rhs=KT_f[prow, ch, k0:k1],
                                         start=True, stop=True)
                    mtmp = sb.tile([P, P], F32, tag="mtmp")
                    nc.vector.tensor_tensor(out=mtmp[:], in0=sc[:, qg * P:kr],
                                            in1=tri[:], op=OP.add)
                    Pb = sb.tile([P, S], BF16, tag="Pb")
                    s2 = sb.tile([P, 1], F32, tag="s2")
                    if qg > 0:
                        s1 = sb.tile([P, 1], F32, tag="s1")
                        nc.scalar.activation(out=Pb[:, 0:qg * P], in_=sc[:, 0:qg * P],
                                             func=AF.Exp, scale=0.125, accum_out=s1[:])
                    nc.scalar.activation(out=Pb[:, qg * P:kr], in_=mtmp[:],
                                         func=AF.Exp, scale=0.125, accum_out=s2[:])
                    den = sb.tile([P, 1], F32, tag="den")
                    if qg > 0:
                        nc.vector.tensor_tensor(out=den[:], in0=s1[:], in1=s2[:],
                                                op=OP.add)
                    else:
                        nc.vector.tensor_copy(out=den[:], in_=s2[:])
                    nc.vector.reciprocal(out=den[:], in_=den[:])
                    diag = sb.tile([P, P], BF16, tag="diag")
                    nc.vector.tensor_tensor(out=diag[:], in0=ident[:],
                                            in1=den[:].to_broadcast([P, P]),
                                            op=OP.mult)
                    at = psa.tile([64, P], F32, tag="sm")
                    for kb in range(qg + 1):
                        ptp = pst.tile([P, P], F32, tag="tp")
                        nc.tensor.matmul(out=ptp[:],
                                         lhsT=Pb[:, kb * P:(kb + 1) * P],
                                         rhs=diag[:], start=True, stop=True)
                        pts = sb.tile([P, P], BF16, tag="pts")
                        nc.scalar.activation(out=pts[:], in_=ptp[:], func=AF.Copy)
                        nc.tensor.matmul(out=at[:],
                                         lhsT=V_f[:, kb, hh * 64:(hh + 1) * 64],
                                         rhs=pts[:], start=(kb == 0), stop=(kb == qg))
                    nc.scalar.activation(out=attnT[prow, ch, qg * P:(qg + 1) * P],
                                         in_=at[:], func=AF.Copy)

            # =========== S5: exchange attn^T, W_O, residual ===========
            xin2 = dram.tile([P, 4 * S], BF16)
            xout2 = dram.tile([2 * P, 4 * S], BF16)
            nc.sync.dma_start(out=xin2[:], in_=attnT[:].rearrange("p c q -> p (c q)"))
            nc.gpsimd.collective_compute("AllGather", OP.bypass,
                                         replica_groups=groups,
                                         ins=[xin2.opt()], outs=[xout2.opt()])
            aT = pp.tile([P, 8, TOK], BF16, tag="at")
            for src in range(2):
                rs = slice(src * P, (src + 1) * P)
                for i in range(4):
                    nc.gpsimd.dma_start(
                        out=aT[:, src * 4 + i, :],
                        in_=xout2[rs, :][:, bass.ds(off512 + i * S, TOK)])
            # scheduler fence: the knowledge_V AllGather queues after the S5
            # collective, overlapping with W_O / S6 / the score phase.
            tc.no_sync_barrier()
            nc.gpsimd.collective_compute("AllGather", OP.bypass,
                                         replica_groups=G8,
                                         ins=[kvb.opt()], outs=[kvo.opt()])
            for t in range(NT):
                ts = slice(t * P, (t + 1) * P)
                po = psb.tile([P, D], F32, tag="big")
                for col in range(2):
                    for ch in range(8):
                        nc.tensor.matmul(out=po[:, col * 512:(col + 1) * 512],
                                         lhsT=aT[:, ch, ts],
                                         rhs=wo[:, ch, col * 512:(col + 1) * 512],
                                         start=(ch == 0), stop=(ch == 7))
                nc.vector.tensor_tensor(out=x_all[:, t, :], in0=po[:],
                                        in1=x_all[:, t, :], op=OP.add)

            # =========== S6: LN2 + compress M -> Qm^T (into hQT) ===========
            for t in range(NT):
                ts = slice(t * P, (t + 1) * P)
                h2 = sb.tile([P, D], BF16, tag="h")
                _ln(nc, sb, x_all[:, t, :], h2[:], eps_t)
                for ch in range(8):
                    tp = pst.tile([P, P], BF16, tag="tp")
                    nc.tensor.transpose(out=tp[:], in_=h2[:, ch * P:(ch + 1) * P],
                                        identity=ident[:])
                    nc.scalar.activation(out=hT[:, ch, ts], in_=tp[:], func=AF.Copy)
                lgm = psa.tile([P, NCMP], F32, tag="sm")
                for ch in range(8):
                    nc.tensor.matmul(out=lgm[:], lhsT=hT[:, ch, ts], rhs=rM[:, ch, :],
                                     start=(ch == 0), stop=(ch == 7))
                wM = sb.tile([P, NCMP], F32, tag="wM")
                _softmax16(nc, sb, lgm[:], wM[:])
                p1a = psb.tile([P, KC], F32, tag="big")
                p1b = psb.tile([P, KC], F32, tag="big")
                for half, pt in ((0, p1a), (1, p1b)):
                    for col in range(2):
                        c0 = half * KC + col * 512
                        for ch in range(8):
                            nc.tensor.matmul(out=pt[:, col * 512:(col + 1) * 512],
                                             lhsT=hT[:, ch, ts],
                                             rhs=neur[:, ch, c0:c0 + 512],
                                             start=(ch == 0), stop=(ch == 7))
                qm = sb.tile([P, R], BF16, tag="hc")
                _combine(nc, sb, (p1a, p1b), wM[:], qm[:])
                tp = pst.tile([P, P], BF16, tag="tp")
                nc.tensor.transpose(out=tp[:], in_=qm[:], identity=ident[:])
                nc.scalar.activation(out=hQT[:, ts], in_=tp[:], func=AF.Copy)

            # =========== S7: knowledge scores, top-8, gather, output ===========
            cands = [pp.tile([P, NKC * 8], U32, tag=f"cand{t}", name=f"cand{t}") for t in range(NT)]
            for c in range(NKC):
                kch8 = gp.tile([P, KC], F8, tag="kch8")
                nc.sync.dma_start(
                    out=kch8[:],
                    in_=kkto[(c // 4) * P:(c // 4 + 1) * P,
                             (c % 4) * KC:(c % 4 + 1) * KC])
                kch = gp.tile([P, KC], BF16, tag="kch")
                nc.scalar.activation(out=kch[:], in_=kch8[:], func=AF.Copy)
                for t in range(NT):
                    ts = slice(t * P, (t + 1) * P)
                    ks = psb.tile([P, KC], F32, tag="big")
                    for col in range(2):
                        nc.tensor.matmul(out=ks[:, col * 512:(col + 1) * 512],
                                         lhsT=hQT[:, ts],
                                         rhs=kch[:, col * 512:(col + 1) * 512],
                                         start=True, stop=True)
                    pk = packed[(c * NT + t) % 3]
                    nc.scalar.activation(out=pk.bitcast(U16)[:, 1::2].bitcast(BF16),
                                         in_=ks[:], func=AF.Copy)
                    c8 = cands[t][:, c * 8:(c + 1) * 8]
                    nc.vector.max(out=c8.bitcast(F32), in_=pk.bitcast(F32)[:])
            for t in range(NT):
                ts = slice(t * P, (t + 1) * P)
                top8 = sb.tile([P, 8], F32, tag="top8")
                nc.vector.max(out=top8[:], in_=cands[t].bitcast(F32)[:])
                pos = sb.tile([P, 8], U32, tag="pos")
                nc.vector.max_index(out=pos[:], in_max=top8[:],
                                    in_values=cands[t].bitcast(F32)[:])
                loc = sb.tile([P, 8], U32, tag="loc")
                nc.vector.tensor_scalar(out=loc[:], in0=top8[:].bitcast(U32),
                                        scalar1=6, scalar2=0x3FF,
                                        op0=OP.logical_shift_right,
                                        op1=OP.bitwise_and)
                cb = sb.tile([P, 8], U32, tag="cb")
                nc.vector.tensor_scalar(out=cb[:], in0=pos[:],
                                        scalar1=3, scalar2=10,
                                        op0=OP.logical_shift_right,
                                        op1=OP.logical_shift_left)
                idx = sb.tile([P, 8], U32, tag="idx")
                nc.vector.tensor_tensor(out=idx[:], in0=cb[:], in1=loc[:],
                                        op=OP.bitwise_or)
                vals = sb.tile([P, 8], F32, tag="vals")
                nc.vector.tensor_scalar(out=vals[:].bitcast(U32),
                                        in0=top8[:].bitcast(U32),
                                        scalar1=0xFFFF0000, scalar2=None,
                                        op0=OP.bitwise_and)
                nc.vector.tensor_scalar_mul(out=vals[:], in0=vals[:],
                                            scalar1=float(SCALE_R / KK_SCALE))
                mx8 = sb.tile([P, 1], F32, tag="mx8")
                nc.vector.tensor_reduce(out=mx8[:], in_=vals[:], axis=AX.X, op=OP.max)
                nmx8 = sb.tile([P, 1], F32, tag="nmx8")
                nc.vector.tensor_scalar_mul(out=nmx8[:], in0=mx8[:], scalar1=-1.0)
                kw = sb.tile([P, 8], F32, tag="kw")
                ks8 = sb.tile([P, 1], F32, tag="ks8")
                nc.scalar.activation(out=kw[:], in_=vals[:], func=AF.Exp,
                                     bias=nmx8[:], scale=1.0, accum_out=ks8[:])
                nc.vector.reciprocal(out=ks8[:], in_=ks8[:])
                # normalize and fold in the fp8 prescale in one instruction
                nc.vector.tensor_scalar(out=kw[:], in0=kw[:], scalar1=ks8[:],
                                        scalar2=float(1.0 / KV_SCALE),
                                        op0=OP.mult, op1=OP.mult)
                acc = sb.tile([P, D], F32, tag="acc")
                for j in range(KK):
                    vg = gp.tile([P, D], F8, tag="vg")
                    nc.gpsimd.indirect_dma_start(
                        out=vg[:], out_offset=None, in_=kvo[:],
                        in_offset=bass.IndirectOffsetOnAxis(ap=idx[:, j:j + 1], axis=0))
                    if j == 0:
                        nc.vector.tensor_scalar(out=acc[:], in0=vg[:],
                                                scalar1=kw[:, 0:1], scalar2=None,
                                                op0=OP.mult)
                    else:
                        nc.vector.scalar_tensor_tensor(out=acc[:], in0=vg[:],
                                                       scalar=kw[:, j:j + 1],
                                                       in1=acc[:], op0=OP.mult,
                                                       op1=OP.add)
                outsb = sb.tile([P, D], BF16, tag="outsb")
                nc.vector.tensor_tensor(out=outsb[:], in0=acc[:],
                                        in1=x_all[:, t, :], op=OP.add)
                nc.sync.dma_start(out=out_t[ts, :], in_=outsb[:])

    nc.finalize()
    return nc


@functools.lru_cache(maxsize=1)
def _get_program():
    return build_program()


def _to_bf16(a):
    """Fast float32 -> bfloat16 with round-to-nearest-even (numpy bit trick)."""
    a = np.ascontiguousarray(a, np.float32)
    v = a.view(np.uint32)
    rounded = v + 0x7FFF + ((v >> 16) & 1)
    return (rounded >> 16).astype(np.uint16).view(ml_dtypes.bfloat16)


def _prep_core_inputs(inputs):
    x = np.asarray(inputs["x"], np.float32)
    neurons = np.asarray(inputs["compress_neurons"], np.float32)
    neur_flat = _to_bf16(neurons.transpose(1, 0, 2).reshape(D, NCMP * R))
    rqkv = _to_bf16(np.concatenate(
        [np.asarray(inputs["router_Q"], np.float32),
         np.asarray(inputs["router_K"], np.float32),
         np.asarray(inputs["router_V"], np.float32)], axis=1))
    rm = _to_bf16(inputs["router_M"])
    wq = _to_bf16(inputs["W_Q"])
    wk = _to_bf16(inputs["W_K"])
    wv = _to_bf16(inputs["W_V"])
    wo = _to_bf16(inputs["W_O"])
    kKT = (np.asarray(inputs["knowledge_K"], np.float32).T * KK_SCALE).astype(
        ml_dtypes.float8_e4m3)
    kV = (np.asarray(inputs["knowledge_V"], np.float32) * KV_SCALE).astype(
        ml_dtypes.float8_e4m3)
    shared = dict(
        tri=np.where(np.tril(np.ones((P, P), bool)), 0.0, NEG).astype(np.float32),
        ident=np.eye(P, dtype=np.float32).astype(ml_dtypes.bfloat16),
    )
    in_maps = []
    for c in range(N_CORES):
        b, hf = c // 2, c % 2
        rs = slice(c * P, (c + 1) * P)
        ws = slice(c * 16, (c + 1) * 16)
        m = dict(shared)
        m["agA"] = np.concatenate([
            rqkv[rs].ravel(), rm[rs].ravel(), wq[ws].ravel(),
            wk[ws].ravel(), wv[ws].ravel(),
            neur_flat[rs, 0:1024].ravel(),
        ]).reshape(1, AGAN)
        m["agB"] = np.ascontiguousarray(
            neur_flat[rs, 1024:2048]).reshape(1, AGBN)
        m["agC"] = np.ascontiguousarray(wo[rs]).reshape(1, AGCN)
        m["kKTsh"] = np.ascontiguousarray(kKT[:, c * NKSH:(c + 1) * NKSH])
        m["kVsh"] = np.ascontiguousarray(kV[c * NKSH:(c + 1) * NKSH, :])
        m["x_shard"] = _to_bf16(x[b, hf * TOK:(hf + 1) * TOK, :])
        m["offs"] = np.array([[hf * 512]], np.uint32)
        in_maps.append(m)
    return in_maps


def kernel(**inputs) -> np.ndarray:
    nc = _get_program()
    in_maps = _prep_core_inputs(inputs)
    res = run_bass_kernel_spmd(nc, in_maps, list(range(N_CORES)))
    out = np.empty((B, S, D), np.float32)
    for c in range(N_CORES):
        b, hf = c // 2, c % 2
        o16 = np.asarray(res.results[c]["out_shard"]).view(np.uint16)
        out[b, hf * TOK:(hf + 1) * TOK, :] = (
            o16.astype(np.uint32) << 16).view(np.float32)
    return out


# revision 46
# speedup vs baseline: 1.0083x; 1.0083x over previous
"""DAWNBlock Trainium2 kernel (8 NeuronCores, SPMD, single NEFF launch).

Sharding: tokens split over cores as (batch b = c//2, seq-half hf = c%2),
512 tokens per core. Attention is sharded by (batch, head-group): after a
pair AllGather of Q^T/K^T/V each core runs causal attention for 8 heads over
the full 1024-token sequence of its batch; a second pair AllGather exchanges
attn^T so each core projects (W_O) only its own 512 tokens. The knowledge
stage is token-parallel: knowledge_K^T is streamed through SBUF in bf16,
scores are computed in 32 chunks of 1024, and top-8 selection uses the
hardware max8 instruction over packed floats (bf16 score in the high 16
bits, global index in the low 15 mantissa bits), followed by per-partition
indirect-DMA row gathers from knowledge_V.

Host->device traffic is minimized by shipping every replicated weight as a
1/8 shard and reassembling on-device with DRAM AllGathers: a small-weights
AllGather (compress neurons, W_O, W_QKV, routers) issued at kernel start,
and knowledge_K^T / knowledge_V AllGathers issued after the attention
collectives so the attention pair-AllGathers don't queue behind the 64MB
knowledge-table gather on the in-order collective queue.

Core-parity-dependent data movement (which half of the AllGather output
belongs to this core) is handled with register-backed dynamic DMA slices
(bass.ds) driven by a tiny per-core offsets input, so all 8 cores share one
instruction stream.
"""
import functools
import numpy as np
import ml_dtypes

import concourse.bass as bass
import concourse.bacc as bacc
import concourse.mybir as mybir
import concourse.tile as tile
from concourse.bass_utils import run_bass_kernel_spmd

F32 = mybir.dt.float32
BF16 = mybir.dt.bfloat16
F8 = mybir.dt.float8e4
U32 = mybir.dt.uint32
U16 = mybir.dt.uint16
KV_SCALE = 64.0        # knowledge_V shipped as fp8e4 * KV_SCALE
KK_SCALE = 64.0        # knowledge_K shipped as fp8e4 * KK_SCALE
AF = mybir.ActivationFunctionType
OP = mybir.AluOpType
AX = mybir.AxisListType

N_CORES = 8
P = 128
D = 1024
R = 128
NCMP = 16
NK = 32768
KK = 8
S = 1024
B = 4
TOK = 512
NT = TOK // P          # 4 token tiles per core
EPS = 1e-5
NEG = -1.0e30
KC = 1024              # knowledge-score chunk width
NKC = NK // KC         # 32 chunks
NKSH = NK // N_CORES   # 4096 knowledge rows per core shard
SCALE_R = float(1.0 / np.sqrt(R))

# small-weights AllGather segment layout (element offsets, bf16).
# Split into three collectives so S1 can start after A (first half of the
# neuron banks), B lands just in time for the second compress half plus
# the attention projections, and W_O (C) gathers during attention. The
# tiny routers ship replicated from the host (128 KB/core).
WX_SZ = 16 * D                  # 16384 (W_Q/W_K/W_V row shard)
NEURH_SZ = P * 1024             # 131072 (half of the neuron banks)
WO_SZ = P * D                   # 131072
AGAN = NEURH_SZ                 # 131072
NB0 = 0
WQ0 = NB0 + NEURH_SZ
WK0 = WQ0 + WX_SZ
WV0 = WK0 + WX_SZ
AGBN = WV0 + WX_SZ              # 180224
AGCN = WO_SZ                    # 131072


def _ln(nc, sb, x_ap, out_ap, eps_tile):
    """LayerNorm (gamma=1, beta=0): x_ap [128, D] f32 -> out_ap (bf16)."""
    stats = sb.tile([P, 2, 6], F32, tag="ln_stats")
    for g in range(2):
        nc.vector.bn_stats(out=stats[:, g, :], in_=x_ap[:, g * 512:(g + 1) * 512])
    mv = sb.tile([P, 2], F32, tag="ln_mv")
    nc.vector.bn_aggr(out=mv[:], in_=stats[:])
    rstd = sb.tile([P, 1], F32, tag="ln_rstd")
    nc.scalar.activation(out=rstd[:], in_=mv[:, 1:2], func=AF.Sqrt,
                         bias=eps_tile[:], scale=1.0)
    nc.vector.reciprocal(out=rstd[:], in_=rstd[:])
    nc.vector.tensor_scalar(out=out_ap, in0=x_ap, scalar1=mv[:, 0:1],
                            scalar2=rstd[:], op0=OP.subtract, op1=OP.mult)


def _softmax16(nc, sb, logits_ap, w_ap):
    """softmax over 16 router logits (PSUM f32 in) -> w_ap [128,16] f32."""
    mx = sb.tile([P, 1], F32, tag="rs_mx")
    nc.vector.tensor_reduce(out=mx[:], in_=logits_ap, axis=AX.X, op=OP.max)
    nmx = sb.tile([P, 1], F32, tag="rs_nmx")
    nc.vector.tensor_scalar_mul(out=nmx[:], in0=mx[:], scalar1=-1.0)
    ssum = sb.tile([P, 1], F32, tag="rs_sum")
    nc.scalar.activation(out=w_ap, in_=logits_ap, func=AF.Exp,
                         bias=nmx[:], scale=1.0, accum_out=ssum[:])
    nc.vector.reciprocal(out=ssum[:], in_=ssum[:])
    nc.vector.tensor_scalar_mul(out=w_ap, in0=w_ap, scalar1=ssum[:])


def _combine(nc, sb, p1_halves, w_ap, out_ap):
    """out[t,:] = sum_n w[t,n] * P1[t, n*128:(n+1)*128] (P1 in 2 PSUM halves)."""
    acc = sb.tile([P, R], F32, tag="cmb_acc")
    for n in range(NCMP):
        src = p1_halves[n // 8][:, (n % 8) * R:(n % 8 + 1) * R]
        if n == 0:
            nc.vector.tensor_scalar(out=acc[:], in0=src, scalar1=w_ap[:, 0:1],
                                    scalar2=None, op0=OP.mult)
        else:
            nc.vector.scalar_tensor_tensor(out=acc[:], in0=src,
                                           scalar=w_ap[:, n:n + 1], in1=acc[:],
                                           op0=OP.mult, op1=OP.add)
    nc.vector.tensor_copy(out=out_ap, in_=acc[:])


def build_program():
    nc = bacc.Bacc(None, num_devices=N_CORES)

    x_in = nc.dram_tensor("x_shard", [TOK, D], BF16, kind="ExternalInput")
    tri_in = nc.dram_tensor("tri", [P, P], F32, kind="ExternalInput")
    id_in = nc.dram_tensor("ident", [P, P], BF16, kind="ExternalInput")
    offs_in = nc.dram_tensor("offs", [1, 1], U32, kind="ExternalInput")
    rqkv_in = nc.dram_tensor("routersQKV", [D, 48], BF16, kind="ExternalInput")
    rm_in = nc.dram_tensor("routerM", [D, NCMP], BF16, kind="ExternalInput")
    aga_in = nc.dram_tensor("agA", [1, AGAN], BF16, kind="ExternalInput")
    agb_in = nc.dram_tensor("agB", [1, AGBN], BF16, kind="ExternalInput")
    agc_in = nc.dram_tensor("agC", [1, AGCN], BF16, kind="ExternalInput")
    kkt_in = nc.dram_tensor("kKTsh", [P, NKSH], F8, kind="ExternalInput")
    kv_in = nc.dram_tensor("kVsh", [NKSH, D], F8, kind="ExternalInput")
    out_t = nc.dram_tensor("out_shard", [TOK, D], BF16, kind="ExternalOutput")

    G8 = [list(range(N_CORES))]

    with tile.TileContext(nc) as tc:
        with (
            tc.tile_pool(name="persist", bufs=1) as pp,
            tc.tile_pool(name="weights", bufs=1) as wp,
            tc.tile_pool(name="work", bufs=2) as sb,
            tc.tile_pool(name="gath", bufs=3) as gp,
            tc.tile_pool(name="ps_big", bufs=2, space="PSUM") as psb,
            tc.tile_pool(name="ps_tp", bufs=2, space="PSUM") as pst,
            tc.tile_pool(name="ps_sm", bufs=2, space="PSUM") as psa,
            tc.tile_pool(name="dram", bufs=1, space="DRAM") as dram,
        ):
            # ---- parity offset -> gpsimd register for dynamic DMA slices ----
            r512 = nc.gpsimd.alloc_register("off512")
            nc.gpsimd.reg_load(r512, offs_in[0:1, 0:1])
            off512 = nc.gpsimd.snap(r512, donate=True, min_val=0, max_val=512)

            # ---- collective staging: bounce shards into DRAM tiles ----
            agab = dram.tile([1, AGAN], BF16)
            agao = dram.tile([N_CORES, AGAN], BF16, addr_space="Shared")
            agbb = dram.tile([1, AGBN], BF16)
            agbo = dram.tile([N_CORES, AGBN], BF16, addr_space="Shared")
            agcb = dram.tile([1, AGCN], BF16)
            agco = dram.tile([N_CORES, AGCN], BF16, addr_space="Shared")
            kktb = dram.tile([P, NKSH], F8)
            kkto = dram.tile([N_CORES * P, NKSH], F8, addr_space="Shared")
            kvb = dram.tile([NKSH, D], F8)
            kvo = dram.tile([NK, D], F8, addr_space="Shared")
            nc.sync.dma_start(out=agab[:], in_=aga_in[:])
            nc.sync.dma_start(out=agbb[:], in_=agb_in[:])
            nc.sync.dma_start(out=agcb[:], in_=agc_in[:])
            # weights gathered in three pieces: S1 needs A (then B); W_O (C)
            # arrives during attention
            nc.gpsimd.collective_compute("AllGather", OP.bypass,
                                         replica_groups=G8,
                                         ins=[agab.opt()], outs=[agao.opt()])
            nc.gpsimd.collective_compute("AllGather", OP.bypass,
                                         replica_groups=G8,
                                         ins=[agbb.opt()], outs=[agbo.opt()])
            nc.gpsimd.collective_compute("AllGather", OP.bypass,
                                         replica_groups=G8,
                                         ins=[agcb.opt()], outs=[agco.opt()])

            # ---- early loads that depend on nothing: masks, routers, x ----
            # (created before the collective-gated weight loads so the
            # in-order SP DMA queue doesn't stall them behind AGb/AGc)
            rQKV = wp.tile([P, 8, 48], BF16)
            nc.sync.dma_start(out=rQKV[:],
                              in_=rqkv_in[:].rearrange("(c p) n -> p c n", p=P))
            rM = wp.tile([P, 8, NCMP], BF16)
            nc.sync.dma_start(out=rM[:],
                              in_=rm_in[:].rearrange("(c p) n -> p c n", p=P))
            ident = wp.tile([P, P], BF16)
            nc.sync.dma_start(out=ident[:], in_=id_in[:])
            tri = wp.tile([P, P], F32)
            nc.sync.dma_start(out=tri[:], in_=tri_in[:])
            eps_t = wp.tile([P, 1], F32)
            nc.vector.memset(eps_t[:], EPS)
            iota_t = wp.tile([P, KC], U16)
            nc.gpsimd.iota(out=iota_t[:], pattern=[[64, KC]], base=0,
                           channel_multiplier=0)
            x_all = pp.tile([P, NT, D], F32)
            for t in range(NT):
                xbf = sb.tile([P, D], BF16, tag="xbf")
                nc.sync.dma_start(out=xbf[:], in_=x_in[t * P:(t + 1) * P, :])
                nc.scalar.activation(out=x_all[:, t, :], in_=xbf[:], func=AF.Copy)
            # knowledge-table shard bounces (consumed much later)
            nc.sync.dma_start(out=kktb[:], in_=kkt_in[:])
            nc.sync.dma_start(out=kvb[:], in_=kv_in[:])

            # ---- resident weights (from the gathered small-weights buffers) ----
            neur = wp.tile([P, 8, NCMP * R], BF16)
            nc.sync.dma_start(
                out=neur[:, :, 0:1024],
                in_=agao[:, :].rearrange("c (p n) -> p c n", p=P))
            nc.sync.dma_start(
                out=neur[:, :, 1024:2048],
                in_=agbo[:, NB0:NB0 + NEURH_SZ].rearrange(
                    "c (p n) -> p c n", p=P))
            wq = wp.tile([P, D], BF16)
            wk = wp.tile([P, D], BF16)
            wv = wp.tile([P, D], BF16)
            for w_, base in ((wq, WQ0), (wk, WK0), (wv, WV0)):
                for c in range(N_CORES):
                    nc.sync.dma_start(
                        out=w_[c * 16:(c + 1) * 16, :],
                        in_=agbo[c:c + 1, base:base + WX_SZ].rearrange(
                            "o (p n) -> (o p) n", p=16))
            wo = wp.tile([P, 8, D], BF16)
            nc.sync.dma_start(
                out=wo[:],
                in_=agco[:, :].rearrange("c (p n) -> p c n", p=P))

            # ---- persistent activations ----
            hT = pp.tile([P, 8, TOK], BF16, tag="hT")
            hQT = pp.tile([P, TOK], BF16, tag="hQT")
            hKT = pp.tile([P, TOK], BF16, tag="hKT")
            hVT = pp.tile([P, TOK], BF16, tag="hVT")

            # packed score buffers (iota pre-written into the low u16 lanes)
            packed = [pp.tile([P, KC], U32, tag=f"pk{i}", name=f"pk{i}") for i in range(3)]
            for pk in packed:
                nc.vector.tensor_copy(out=pk.bitcast(U16)[:, 0::2], in_=iota_t[:])

            # =========== S1: LN1, shared projection, routed compress ===========
            for t in range(NT):
                ts = slice(t * P, (t + 1) * P)
                h = sb.tile([P, D], BF16, tag="h")
                _ln(nc, sb, x_all[:, t, :], h[:], eps_t)
                for ch in range(8):
                    tp = pst.tile([P, P], BF16, tag="tp")
                    nc.tensor.transpose(out=tp[:], in_=h[:, ch * P:(ch + 1) * P],
                                        identity=ident[:])
                    nc.scalar.activation(out=hT[:, ch, ts], in_=tp[:], func=AF.Copy)
                lg = psa.tile([P, 48], F32, tag="sm")
                for ch in range(8):
                    nc.tensor.matmul(out=lg[:], lhsT=hT[:, ch, ts], rhs=rQKV[:, ch, :],
                                     start=(ch == 0), stop=(ch == 7))
                wQKV = sb.tile([P, 48], F32, tag="wQKV")
                for rr in range(3):
                    _softmax16(nc, sb, lg[:, rr * 16:(rr + 1) * 16],
                               wQKV[:, rr * 16:(rr + 1) * 16])
                p1a = psb.tile([P, KC], F32, tag="big")
                p1b = psb.tile([P, KC], F32, tag="big")
                for half, pt in ((0, p1a), (1, p1b)):
                    for col in range(2):
                        c0 = half * KC + col * 512
                        for ch in range(8):
                            nc.tensor.matmul(out=pt[:, col * 512:(col + 1) * 512],
                                             lhsT=hT[:, ch, ts],
                                             rhs=neur[:, ch, c0:c0 + 512],
                                             start=(ch == 0), stop=(ch == 7))
                for rr, dst in ((0, hQT), (1, hKT), (2, hVT)):
                    hc = sb.tile([P, R], BF16, tag="hc")
                    _combine(nc, sb, (p1a, p1b), wQKV[:, rr * 16:(rr + 1) * 16], hc[:])
                    tp = pst.tile([P, P], BF16, tag="tp")
                    nc.tensor.transpose(out=tp[:], in_=hc[:], identity=ident[:])
                    nc.scalar.activation(out=dst[:, ts], in_=tp[:], func=AF.Copy)

            # =========== S3: pair AllGather of h-compressions hQ/hK/hV^T ======
            # Exchange the rank-128 compressed activations (0.38 MB) instead
            # of the projected Q^T/K^T/V (3 MB); both halves' projections are
            # recomputed locally (a few matmuls). The head-half selection is
            # done once by dynamically slicing W_Q/W_K/W_V columns with the
            # parity register, keeping one SPMD instruction stream.
            groups = [[0, 1], [2, 3], [4, 5], [6, 7]]
            xin = dram.tile([P, 1536], BF16)
            xout = dram.tile([2 * P, 1536], BF16)
            nc.sync.dma_start(out=xin[:, 0:512], in_=hQT[:])
            nc.sync.dma_start(out=xin[:, 512:1024], in_=hKT[:])
            nc.sync.dma_start(out=xin[:, 1024:1536], in_=hVT[:])
            nc.gpsimd.collective_compute("AllGather", OP.bypass,
                                         replica_groups=groups,
                                         ins=[xin.opt()], outs=[xout.opt()])
            # parity-sliced projection weights (my 4 head-chunks = 512 cols)
            wq_my = wp.tile([P, TOK], BF16)
            wk_my = wp.tile([P, TOK], BF16)
            wv_my = wp.tile([P, TOK], BF16)
            for w_, wm in ((wq, wq_my), (wk, wk_my), (wv, wv_my)):
                nc.gpsimd.dma_start(out=wm[:],
                                    in_=w_[:][:, bass.ds(off512, TOK)])
            # full-sequence h-compressions (row block s = seq half s)
            hQf = pp.tile([P, S], BF16, tag="hQf")
            hKf = pp.tile([P, S], BF16, tag="hKf")
            hVf = pp.tile([P, S], BF16, tag="hVf")
            for s in range(2):
                rs = slice(s * P, (s + 1) * P)
                sl = slice(s * TOK, (s + 1) * TOK)
                nc.sync.dma_start(out=hQf[:, sl], in_=xout[rs, 0:512])
                nc.sync.dma_start(out=hKf[:, sl], in_=xout[rs, 512:1024])
                nc.sync.dma_start(out=hVf[:, sl], in_=xout[rs, 1024:1536])
            # project both halves for my heads only
            QT_f = pp.tile([P, 4, S], BF16, tag="qt")
            KT_f = pp.tile([P, 4, S], BF16, tag="kt")
            V_f = pp.tile([P, 8, 512], BF16, tag="vv")
            for wm, hsrc, dst in ((wq_my, hQf, QT_f), (wk_my, hKf, KT_f)):
                for i in range(4):
                    pr = psb.tile([P, S], F32, tag="big")
                    for col in range(2):
                        nc.tensor.matmul(out=pr[:, col * 512:(col + 1) * 512],
                                         lhsT=wm[:, i * P:(i + 1) * P],
                                         rhs=hsrc[:, col * 512:(col + 1) * 512],
                                         start=True, stop=True)
                    nc.scalar.activation(out=dst[:, i, :], in_=pr[:], func=AF.Copy)
            for blk in range(8):
                pv = pst.tile([P, TOK], F32, tag="tp")
                nc.tensor.matmul(out=pv[:], lhsT=hVf[:, blk * P:(blk + 1) * P],
                                 rhs=wv_my[:], start=True, stop=True)
                nc.scalar.activation(out=V_f[:, blk, :], in_=pv[:], func=AF.Copy)

            # scheduler fence: the kKT AllGather may not be hoisted before the
            # S3 collective above; it then fills the collective-engine gap
            # while attention computes.
            tc.no_sync_barrier()
            nc.gpsimd.collective_compute("AllGather", OP.bypass,
                                         replica_groups=G8,
                                         ins=[kktb.opt()], outs=[kkto.opt()])

            # =========== S4: causal attention, 8 heads, full sequence ===========
            attnT = pp.tile([P, 4, S], BF16, tag="at")
            for hh in range(8):
                ch, poff = hh // 2, (hh % 2) * 64
                prow = slice(poff, poff + 64)
                for qg in range(8):
                    kr = (qg + 1) * P
                    sc = psb.tile([P, S], F32, tag="big")
                    for part in range((kr + 511) // 512):
                        k0, k1 = part * 512, min(kr, (part + 1) * 512)
                        nc.tensor.matmul(out=sc[:, k0:k1],
                                         lhsT=QT_f[prow, ch, qg * P:(qg + 1) * P],
                                         rhs=KT_f[prow, ch, k0:k1],
                                         start=True, stop=True)
                    mtmp = sb.tile([P, P], F32, tag="mtmp")
                    nc.vector.tensor_tensor(out=mtmp[:], in0=sc[:, qg * P:kr],
                                            in1=tri[:], op=OP.add)
                    Pb = sb.tile([P, S], BF16, tag="Pb")
                    s2 = sb.tile([P, 1], F32, tag="s2")
                    if qg > 0:
                        s1 = sb.tile([P, 1], F32, tag="s1")
                        nc.scalar.activation(out=Pb[:, 0:qg * P], in_=sc[:, 0:qg * P],
                                             func=AF.Exp, scale=0.125, accum_out=s1[:])
                    nc.scalar.activation(out=Pb[:, qg * P:kr], in_=mtmp[:],
                                         func=AF.Exp, scale=0.125, accum_out=s2[:])
                    den = sb.tile([P, 1], F32, tag="den")
                    if qg > 0:
                        nc.vector.tensor_tensor(out=den[:], in0=s1[:], in1=s2[:],
                                                op=OP.add)
                    else:
                        nc.vector.tensor_copy(out=den[:], in_=s2[:])
                    nc.vector.reciprocal(out=den[:], in_=den[:])
                    diag = sb.tile([P, P], BF16, tag="diag")
                    nc.vector.tensor_tensor(out=diag[:], in0=ident[:],
                                            in1=den[:].to_broadcast([P, P]),
                                            op=OP.mult)
                    at = psa.tile([64, P], F32, tag="sm")
                    for kb in range(qg + 1):
                        ptp = pst.tile([P, P], F32, tag="tp")
                        nc.tensor.matmul(out=ptp[:],
                                         lhsT=Pb[:, kb * P:(kb + 1) * P],
                                         rhs=diag[:], start=True, stop=True)
                        pts = sb.tile([P, P], BF16, tag="pts")
                        nc.vector.tensor_copy(out=pts[:], in_=ptp[:])
                        nc.tensor.matmul(out=at[:],
                                         lhsT=V_f[:, kb, hh * 64:(hh + 1) * 64],
                                         rhs=pts[:], start=(kb == 0), stop=(kb == qg))
                    nc.scalar.activation(out=attnT[prow, ch, qg * P:(qg + 1) * P],
                                         in_=at[:], func=AF.Copy)

            # =========== S5: exchange attn^T, W_O, residual ===========
            xin2 = dram.tile([P, 4 * S], BF16)
            xout2 = dram.tile([2 * P, 4 * S], BF16)
            nc.sync.dma_start(out=xin2[:], in_=attnT[:].rearrange("p c q -> p (c q)"))
            nc.gpsimd.collective_compute("AllGather", OP.bypass,
                                         replica_groups=groups,
                                         ins=[xin2.opt()], outs=[xout2.opt()])
            aT = pp.tile([P, 8, TOK], BF16, tag="at")
            for src in range(2):
                rs = slice(src * P, (src + 1) * P)
                for i in range(4):
                    nc.gpsimd.dma_start(
                        out=aT[:, src * 4 + i, :],
                        in_=xout2[rs, :][:, bass.ds(off512 + i * S, TOK)])
            # scheduler fence: the knowledge_V AllGather queues after the S5
            # collective, overlapping with W_O / S6 / the score phase.
            tc.no_sync_barrier()
            nc.gpsimd.collective_compute("AllGather", OP.bypass,
                                         replica_groups=G8,
                                         ins=[kvb.opt()], outs=[kvo.opt()])
            for t in range(NT):
                ts = slice(t * P, (t + 1) * P)
                po = psb.tile([P, D], F32, tag="big")
                for col in range(2):
                    for ch in range(8):
                        nc.tensor.matmul(out=po[:, col * 512:(col + 1) * 512],
                                         lhsT=aT[:, ch, ts],
                                         rhs=wo[:, ch, col * 512:(col + 1) * 512],
                                         start=(ch == 0), stop=(ch == 7))
                nc.vector.tensor_tensor(out=x_all[:, t, :], in0=po[:],
                                        in1=x_all[:, t, :], op=OP.add)

            # =========== S6: LN2 + compress M -> Qm^T (into hQT) ===========
            for t in range(NT):
                ts = slice(t * P, (t + 1) * P)
                h2 = sb.tile([P, D], BF16, tag="h")
                _ln(nc, sb, x_all[:, t, :], h2[:], eps_t)
                for ch in range(8):
                    tp = pst.tile([P, P], BF16, tag="tp")
                    nc.tensor.transpose(out=tp[:], in_=h2[:, ch * P:(ch + 1) * P],
                                        identity=ident[:])
                    nc.scalar.activation(out=hT[:, ch, ts], in_=tp[:], func=AF.Copy)
                lgm = psa.tile([P, NCMP], F32, tag="sm")
                for ch in range(8):
                    nc.tensor.matmul(out=lgm[:], lhsT=hT[:, ch, ts], rhs=rM[:, ch, :],
                                     start=(ch == 0), stop=(ch == 7))
                wM = sb.tile([P, NCMP], F32, tag="wM")
                _softmax16(nc, sb, lgm[:], wM[:])
                p1a = psb.tile([P, KC], F32, tag="big")
                p1b = psb.tile([P, KC], F32, tag="big")
                for half, pt in ((0, p1a), (1, p1b)):
                    for col in range(2):
                        c0 = half * KC + col * 512
                        for ch in range(8):
                            nc.tensor.matmul(out=pt[:, col * 512:(col + 1) * 512],
                                             lhsT=hT[:, ch, ts],
                                             rhs=neur[:, ch, c0:c0 + 512],
                                             start=(ch == 0), stop=(ch == 7))
                qm = sb.tile([P, R], BF16, tag="hc")
                _combine(nc, sb, (p1a, p1b), wM[:], qm[:])
                tp = pst.tile([P, P], BF16, tag="tp")
                nc.tensor.transpose(out=tp[:], in_=qm[:], identity=ident[:])
                nc.scalar.activation(out=hQT[:, ts], in_=tp[:], func=AF.Copy)

            # =========== S7: knowledge scores, top-8, gather, output ===========
            cands = [pp.tile([P, NKC * 8], U32, tag=f"cand{t}", name=f"cand{t}") for t in range(NT)]
            for c in range(NKC):
                kch8 = gp.tile([P, KC], F8, tag="kch8")
                nc.sync.dma_start(
                    out=kch8[:],
                    in_=kkto[(c // 4) * P:(c // 4 + 1) * P,
                             (c % 4) * KC:(c % 4 + 1) * KC])
                for t in range(NT):
                    ts = slice(t * P, (t + 1) * P)
                    ks = psb.tile([P, KC], F32, tag="big")
                    for col in range(2):
                        nc.tensor.matmul(out=ks[:, col * 512:(col + 1) * 512],
                                         lhsT=hQT[:, ts],
                                         rhs=kch8[:, col * 512:(col + 1) * 512],
                                         start=True, stop=True)
                    pk = packed[(c * NT + t) % 3]
                    nc.scalar.activation(out=pk.bitcast(U16)[:, 1::2].bitcast(BF16),
                                         in_=ks[:], func=AF.Copy)
                    c8 = cands[t][:, c * 8:(c + 1) * 8]
                    nc.vector.max(out=c8.bitcast(F32), in_=pk.bitcast(F32)[:])
            for t in range(NT):
                ts = slice(t * P, (t + 1) * P)
                top8 = sb.tile([P, 8], F32, tag="top8")
                nc.vector.max(out=top8[:], in_=cands[t].bitcast(F32)[:])
                pos = sb.tile([P, 8], U32, tag="pos")
                nc.vector.max_index(out=pos[:], in_max=top8[:],
                                    in_values=cands[t].bitcast(F32)[:])
                loc = sb.tile([P, 8], U32, tag="loc")
                nc.vector.tensor_scalar(out=loc[:], in0=top8[:].bitcast(U32),
                                        scalar1=6, scalar2=0x3FF,
                                        op0=OP.logical_shift_right,
                                        op1=OP.bitwise_and)
                cb = sb.tile([P, 8], U32, tag="cb")
                nc.vector.tensor_scalar(out=cb[:], in0=pos[:],
                                        scalar1=3, scalar2=10,
                                        op0=OP.logical_shift_right,
                                        op1=OP.logical_shift_left)
                idx = sb.tile([P, 8], U32, tag="idx")
                nc.vector.tensor_tensor(out=idx[:], in0=cb[:], in1=loc[:],
                                        op=OP.bitwise_or)
                vals = sb.tile([P, 8], F32, tag="vals")
                nc.vector.tensor_scalar(out=vals[:].bitcast(U32),
                                        in0=top8[:].bitcast(U32),
                                        scalar1=0xFFFF0000, scalar2=None,
                                        op0=OP.bitwise_and)
                nc.vector.tensor_scalar_mul(out=vals[:], in0=vals[:],
                                            scalar1=float(SCALE_R / KK_SCALE))
                mx8 = sb.tile([P, 1], F32, tag="mx8")
                nc.vector.tensor_reduce(out=mx8[:], in_=vals[:], axis=AX.X, op=OP.max)
                nmx8 = sb.tile([P, 1], F32, tag="nmx8")
                nc.vector.tensor_scalar_mul(out=nmx8[:], in0=mx8[:], scalar1=-1.0)
                kw = sb.tile([P, 8], F32, tag="kw")
                ks8 = sb.tile([P, 1], F32, tag="ks8")
                nc.scalar.activation(out=kw[:], in_=vals[:], func=AF.Exp,
                                     bias=nmx8[:], scale=1.0, accum_out=ks8[:])
                nc.vector.reciprocal(out=ks8[:], in_=ks8[:])
                # normalize and fold in the fp8 prescale in one instruction
                nc.vector.tensor_scalar(out=kw[:], in0=kw[:], scalar1=ks8[:],
                                        scalar2=float(1.0 / KV_SCALE),
                                        op0=OP.mult, op1=OP.mult)
                acc = sb.tile([P, D], F32, tag="acc")
                for j in range(KK):
                    vg = gp.tile([P, D], F8, tag="vg")
                    nc.gpsimd.indirect_dma_start(
                        out=vg[:], out_offset=None, in_=kvo[:],
                        in_offset=bass.IndirectOffsetOnAxis(ap=idx[:, j:j + 1], axis=0))
                    if j == 0:
                        nc.vector.tensor_scalar(out=acc[:], in0=vg[:],
                                                scalar1=kw[:, 0:1], scalar2=None,
                                                op0=OP.mult)
                    else:
                        nc.vector.scalar_tensor_tensor(out=acc[:], in0=vg[:],
                                                       scalar=kw[:, j:j + 1],
                                                       in1=acc[:], op0=OP.mult,
                                                       op1=OP.add)
                outsb = sb.tile([P, D], BF16, tag="outsb")
                nc.vector.tensor_tensor(out=outsb[:], in0=acc[:],
                                        in1=x_all[:, t, :], op=OP.add)
                nc.sync.dma_start(out=out_t[ts, :], in_=outsb[:])

    nc.finalize()
    return nc


@functools.lru_cache(maxsize=1)
def _get_program():
    return build_program()


def _to_bf16(a):
    """Fast float32 -> bfloat16 with round-to-nearest-even (numpy bit trick)."""
    a = np.ascontiguousarray(a, np.float32)
    v = a.view(np.uint32)
    rounded = v + 0x7FFF + ((v >> 16) & 1)
    return (rounded >> 16).astype(np.uint16).view(ml_dtypes.bfloat16)


def _prep_core_inputs(inputs):
    x = np.asarray(inputs["x"], np.float32)
    neurons = np.asarray(inputs["compress_neurons"], np.float32)
    neur_flat = _to_bf16(neurons.transpose(1, 0, 2).reshape(D, NCMP * R))
    rqkv = _to_bf16(np.concatenate(
        [np.asarray(inputs["router_Q"], np.float32),
         np.asarray(inputs["router_K"], np.float32),
         np.asarray(inputs["router_V"], np.float32)], axis=1))
    rm = _to_bf16(inputs["router_M"])
    wq = _to_bf16(inputs["W_Q"])
    wk = _to_bf16(inputs["W_K"])
    wv = _to_bf16(inputs["W_V"])
    wo = _to_bf16(inputs["W_O"])
    kKT = (np.asarray(inputs["knowledge_K"], np.float32).T * KK_SCALE).astype(
        ml_dtypes.float8_e4m3)
    kV = (np.asarray(inputs["knowledge_V"], np.float32) * KV_SCALE).astype(
        ml_dtypes.float8_e4m3)
    shared = dict(
        tri=np.where(np.tril(np.ones((P, P), bool)), 0.0, NEG).astype(np.float32),
        ident=np.eye(P, dtype=np.float32).astype(ml_dtypes.bfloat16),
        routersQKV=np.ascontiguousarray(rqkv),
        routerM=np.ascontiguousarray(rm),
    )
    in_maps = []
    for c in range(N_CORES):
        b, hf = c // 2, c % 2
        rs = slice(c * P, (c + 1) * P)
        ws = slice(c * 16, (c + 1) * 16)
        m = dict(shared)
        m["agA"] = np.ascontiguousarray(
            neur_flat[rs, 0:1024]).reshape(1, AGAN)
        m["agB"] = np.concatenate([
            neur_flat[rs, 1024:2048].ravel(), wq[ws].ravel(),
            wk[ws].ravel(), wv[ws].ravel(),
        ]).reshape(1, AGBN)
        m["agC"] = np.ascontiguousarray(wo[rs]).reshape(1, AGCN)
        m["kKTsh"] = np.ascontiguousarray(kKT[:, c * NKSH:(c + 1) * NKSH])
        m["kVsh"] = np.ascontiguousarray(kV[c * NKSH:(c + 1) * NKSH, :])
        m["x_shard"] = _to_bf16(x[b, hf * TOK:(hf + 1) * TOK, :])
        m["offs"] = np.array([[hf * 512]], np.uint32)
        in_maps.append(m)
    return in_maps


def kernel(**inputs) -> np.ndarray:
    nc = _get_program()
    in_maps = _prep_core_inputs(inputs)
    res = run_bass_kernel_spmd(nc, in_maps, list(range(N_CORES)))
    out = np.empty((B, S, D), np.float32)
    for c in range(N_CORES):
        b, hf = c // 2, c % 2
        o16 = np.asarray(res.results[c]["out_shard"]).view(np.uint16)
        out[b, hf * TOK:(hf + 1) * TOK, :] = (
            o16.astype(np.uint32) << 16).view(np.float32)
    return out
